# revision 18
# baseline (speedup 1.0000x reference)
"""Sparse-attention kernel for trn2, data-parallel over batch on 8 NeuronCores.

Problem (hardcoded): x:(64,528,768) f32, Wq/Wk/Wv/Wp:(768,768), bp:(768,).
L = 528 tokens = 128 template/online-template tokens + 400 search tokens.
Queries 0:128 attend to keys 0:128; queries 128:528 attend to all 528 keys.
12 heads of dim 64, scale = 768**-0.5, out = softmax(qk^T*scale)v @ Wp + bp.

Sharding: batch 64 -> 8 cores x 8 batches. No collectives.

Device strategy (per core, per batch):
  - host pre-transposes x to xT (d-major) and pre-casts inputs to bf16
  - QT/KT GEMMs produce d-major [768, 528] activations
  - V GEMM writes token-major V into a per-head 128-col stationary block:
      even head h: [ V(64) | ones(64) ]
      odd  head h: [ ones(64) | V(64) ]
    so one PV matmul per head emits O at the head's home lanes (0:64 for
    even, 64:128 for odd, matching its rows in the merged d-major OT tile)
    plus 64 redundant copies of the softmax sums at the opposite lanes --
    sums cost zero extra PE time and no cross-partition moves.
  - scores are computed transposed (S^T[t, l]) per head-pair with PE row
    tiling (two K=64 matmuls on row groups 0:64 / 64:128 run concurrently)
  - exp on ScalarE with the 1/sqrt(768) scale fused into the activation.
    Max-subtraction is skipped: scores are O(0.1) for this problem's
    distribution, exp is exact there, and softmax is shift-invariant.
  - normalization: DVE reciprocal straight from the PSUM sums rows,
    gpsimd partition_broadcast to the head's lanes, DVE mul into OT (bf16)
  - projection GEMM -> Y^T, cast to bf16 in SBUF, DMA out; host transposes
    back and adds bp (zeros per spec, applied host-side for generality).
"""

import numpy as np
import ml_dtypes

import concourse.bass as bass
import concourse.mybir as mybir
import concourse.tile as tile
from concourse.bass_utils import run_bass_kernel_spmd

# ---- problem constants ------------------------------------------------------
B, L, D, H, DH = 64, 528, 768, 12, 64
NCORES = 8
BPC = B // NCORES          # batches per core
ND = D // 128              # 6 d-tiles
NT = (L + 127) // 128      # 5 token tiles (4x128 + 16)
TTAIL = L - 4 * 128        # 16
LA = 128                   # part-A queries (and keys)
LS = L - LA                # 400 part-B (search) queries
NP = H // 2                # 6 head pairs
SCALE = float(D) ** -0.5

BF = mybir.dt.bfloat16
F32 = mybir.dt.float32


def _split_multi_waits(nc, max_waits=1):
    """walrus in this environment rejects instructions carrying more than
    one sync-wait command.  Tile's scheduler freely attaches several.  Hoist
    the extras onto dedicated same-engine NOPs emitted just before the
    instruction (engine streams execute a block in order, so the semantics
    are identical)."""
    n_split = 0
    for fn in nc.m.functions:
        for bb in fn.blocks:
            insts = list(bb.instructions)
            if not any(
                getattr(i, "sync_info", None) is not None
                and len(i.sync_info.on_wait) > max_waits
                for i in insts
            ):
                continue
            out = []
            for inst in insts:
                si = getattr(inst, "sync_info", None)
                if si is not None and len(si.on_wait) > max_waits:
                    waits = list(si.on_wait)
                    for w in waits[:-max_waits]:
                        nop = mybir.InstNoOp(
                            name=f"WS-{nc.next_id()}",
                            engine=inst.engine,
                            sync_info=mybir.SyncInfo(on_wait=[w], on_update=[]),
                            bass_nofuse=True,
                        )
                        nc.register_instruction(nop, overwrite=True)
                        out.append(nop)
                    inst.sync_info = mybir.SyncInfo(
                        on_wait=waits[-max_waits:], on_update=list(si.on_update)
                    )
                    n_split += 1
                out.append(inst)
            bb.instructions = out
    return n_split


def _tp(t):
    """token-partition count of token tile t (last tile is a 16-row tail)"""
    return 128 if t < NT - 1 else TTAIL


def build_bass(bpc=BPC, split_waits=True):
    nc = bass.Bass()
    xt_ext = nc.declare_dram_parameter("xt", [bpc, D, L], BF, isOutput=False)
    w_ext = {
        n: nc.declare_dram_parameter(n, [D, D], BF, isOutput=False)
        for n in ("wq", "wk", "wv", "wp")
    }
    yt_ext = nc.declare_dram_parameter("yt", [bpc, D, L], BF, isOutput=True)

    with tile.TileContext(nc) as tc:
        with (
            tc.tile_pool(name="const", bufs=1) as constp,
            tc.tile_pool(name="xt", bufs=2) as xtp,
            tc.tile_pool(name="qt", bufs=2) as qtp,
            tc.tile_pool(name="kt", bufs=2) as ktp,
            tc.tile_pool(name="et", bufs=2) as etp,
            tc.tile_pool(name="eta", bufs=2) as etap,
            tc.tile_pool(name="ot", bufs=2) as otp,
            tc.tile_pool(name="rst", bufs=3) as rstp,
            tc.tile_pool(name="rbc", bufs=3) as rbcp,
            tc.tile_pool(name="yst", bufs=3) as ystp,
            # PSUM budget is 8 banks, statically reserved per pool:
            # mm 1-bank x2, st 2-bank x1, o 2-bank x1, A(shared sta/oa) 1, y 1
            tc.tile_pool(name="ps_mm", bufs=2, space="PSUM") as psmm,
            tc.tile_pool(name="ps_st", bufs=1, space="PSUM") as psst,
            tc.tile_pool(name="ps_o", bufs=1, space="PSUM") as pso,
            tc.tile_pool(name="ps_a", bufs=1, space="PSUM") as psa,
            tc.tile_pool(name="ps_y", bufs=1, space="PSUM") as psy,
        ):
            # ---- weights, k-tile-major: [128, k_tile, dout] -----------------
            w_sb = {}
            for n in ("wq", "wk", "wv", "wp"):
                t = constp.tile([128, ND, D], BF, tag=n)
                nc.sync.dma_start(t[:], w_ext[n].rearrange("(n p) m -> p n m", p=128))
                w_sb[n] = t

            # ---- static V-block tiles (double buffered manually) ------------
            # layout [128 tokens, NT, NP, parity, 128]:
            #   parity 0 (even head): cols 0:64 V,    cols 64:128 ones
            #   parity 1 (odd head):  cols 0:64 ones, cols 64:128 V
            vz_tiles = []
            for i in range(2):
                v = constp.tile([128, NT, NP, 2, 128], BF, tag=f"vz{i}")
                nc.gpsimd.memset(v[:, :, :, 0, 64:128], 1.0)
                nc.gpsimd.memset(v[:, :, :, 1, 0:64], 1.0)
                vz_tiles.append(v)

            for b in range(bpc):
                vz = vz_tiles[b % 2]

                # ---- load xT(b): [128, k_tile, L] ---------------------------
                xt = xtp.tile([128, ND, L], BF)
                nc.sync.dma_start(
                    xt[:], xt_ext[b].rearrange("(n p) m -> p n m", p=128)
                )

                # ---- QT / KT GEMMs (d-major out) ----------------------------
                qt = qtp.tile([128, ND, L], BF)
                kt = ktp.tile([128, ND, L], BF)
                for dst, wname in ((qt, "wq"), (kt, "wk")):
                    w = w_sb[wname]
                    for m in range(ND):
                        lhsTs = w[:, :, m * 128:(m + 1) * 128]
                        for c in range(2):
                            ps = psmm.tile([128, 512], F32, tag="mm")
                            for k in range(ND):
                                nc.tensor.matmul(
                                    ps[:, 0:264], lhsTs[:, k, :],
                                    xt[:, k, c * 264:(c + 1) * 264],
                                    start=(k == 0), stop=(k == ND - 1),
                                )
                            # cast psum f32 -> sbuf bf16
                            nc.vector.tensor_copy(
                                dst[:, m, c * 264:(c + 1) * 264], ps[:, 0:264]
                            )

                # ---- V GEMM (token-major into vz blocks) --------------------
                wv = w_sb["wv"]
                for t in range(NT):
                    tp = _tp(t)
                    for c in range(2):
                        ps = psmm.tile([128, 512], F32, tag="mm")
                        for k in range(ND):
                            nc.tensor.matmul(
                                ps[0:tp, 0:384], xt[:, k, t * 128:t * 128 + tp],
                                wv[:, k, c * 384:(c + 1) * 384],
                                start=(k == 0), stop=(k == ND - 1),
                            )
                        # scatter-cast: chunk c = heads 6c..6c+5 = pairs 3c..3c+2
                        p0 = 3 * c
                        chunk = ps[0:tp, 0:384].rearrange(
                            "p (pr q n) -> p pr q n", pr=3, q=2
                        )
                        # even heads -> parity 0, cols 0:64
                        nc.vector.tensor_copy(
                            vz[0:tp, t, p0:p0 + 3, 0, 0:64], chunk[:, :, 0, :]
                        )
                        # odd heads -> parity 1, cols 64:128
                        nc.vector.tensor_copy(
                            vz[0:tp, t, p0:p0 + 3, 1, 64:128], chunk[:, :, 1, :]
                        )

                # ---- attention, per head pair -------------------------------
                ot = otp.tile([128, ND, L], BF)

                for p in range(NP):
                    # part B scores S^T[t, l] for both heads (row-tiled pair)
                    et = etp.tile([128, NT, 2, LS], BF)
                    for t in range(NT):
                        tp = _tp(t)
                        stp = psst.tile([128, 2, 512], F32, tag="st")
                        nc.tensor.matmul(
                            stp[0:tp, 0, 0:LS],
                            kt[0:64, p, t * 128:t * 128 + tp],
                            qt[0:64, p, LA:L],
                            tile_position=(0, 0),
                        )
                        nc.tensor.matmul(
                            stp[0:tp, 1, 0:LS],
                            kt[64:128, p, t * 128:t * 128 + tp],
                            qt[64:128, p, LA:L],
                            tile_position=(64, 0),
                        )
                        nc.scalar.activation(
                            et[0:tp, t, :, :], stp[0:tp, :, 0:LS],
                            mybir.ActivationFunctionType.Exp, scale=SCALE,
                        )

                    # part A scores (keys 0:128, queries 0:128).  The two
                    # row-tiled matmuls run concurrently, so they must land
                    # in different PSUM banks (same-bank concurrent PE writes
                    # are a hardware fault).
                    sta = psst.tile([128, 2, 512], F32, tag="st")
                    eta = etap.tile([128, 2, LA], BF)
                    nc.tensor.matmul(
                        sta[:, 0, 0:LA], kt[0:64, p, 0:LA], qt[0:64, p, 0:LA],
                        tile_position=(0, 0),
                    )
                    nc.tensor.matmul(
                        sta[:, 1, 0:LA], kt[64:128, p, 0:LA],
                        qt[64:128, p, 0:LA], tile_position=(64, 0),
                    )
                    nc.scalar.activation(
                        eta[:], sta[:, :, 0:LA],
                        mybir.ActivationFunctionType.Exp, scale=SCALE,
                    )

                    # EV part B: accumulate over token tiles.
                    # even head (j=0): O at rows 0:64, sums copies at 64:128
                    # odd  head (j=1): sums copies at rows 0:64, O at 64:128
                    ops = pso.tile([128, 2, 512], F32, tag="o")
                    for j in range(2):
                        for t in range(NT):
                            tp = _tp(t)
                            nc.tensor.matmul(
                                ops[:, j, 0:LS],
                                vz[0:tp, t, p, j, :],
                                et[0:tp, t, j, :],
                                start=(t == 0), stop=(t == NT - 1),
                            )
                    # EV part A (keys tile 0 only)
                    oa = psa.tile([128, 512], F32, tag="a")
                    nc.tensor.matmul(oa[:, 0:LA], vz[:, 0, p, 0, :], eta[:, 0, :])
                    nc.tensor.matmul(
                        oa[:, LA:2 * LA], vz[:, 0, p, 1, :], eta[:, 1, :]
                    )

                    # reciprocal of the sums, straight from PSUM (multi-lane:
                    # sums rows are replicated 64x by the ones columns)
                    rst = rstp.tile([128, L], F32)
                    nc.vector.reciprocal(
                        rst[64:128, LA:L], ops[64:128, 0, 0:LS]
                    )
                    nc.vector.reciprocal(
                        rst[0:64, LA:L], ops[0:64, 1, 0:LS]
                    )
                    nc.vector.reciprocal(
                        rst[64:128, 0:LA], oa[64:128, 0:LA]
                    )
                    nc.vector.reciprocal(
                        rst[0:64, 0:LA], oa[0:64, LA:2 * LA]
                    )

                    # the ones-columns already replicated the sums across 64
                    # lanes; swap the halves so each head's recip lands on its
                    # home lanes (plain strided SBUF->SBUF DMA)
                    rbc = rbcp.tile([128, L], F32)
                    nc.sync.dma_start(rbc[0:64, :], rst[64:128, :])
                    nc.sync.dma_start(rbc[64:128, :], rst[0:64, :])

                    # scale into the merged d-major OT tile (bf16)
                    nc.vector.tensor_mul(
                        ot[0:64, p, LA:L], ops[0:64, 0, 0:LS], rbc[0:64, LA:L]
                    )
                    nc.vector.tensor_mul(
                        ot[64:128, p, LA:L], ops[64:128, 1, 0:LS],
                        rbc[64:128, LA:L],
                    )
                    nc.vector.tensor_mul(
                        ot[0:64, p, 0:LA], oa[0:64, 0:LA], rbc[0:64, 0:LA]
                    )
                    nc.vector.tensor_mul(
                        ot[64:128, p, 0:LA], oa[64:128, LA:2 * LA],
                        rbc[64:128, 0:LA],
                    )

                # ---- projection: Y^T = Wp^T @ OT -> bf16 -> DRAM ------------
                wp = w_sb["wp"]
                for m in range(ND):
                    for c in range(2):
                        yp = psy.tile([128, 264], F32, tag="y")
                        for k in range(ND):
                            nc.tensor.matmul(
                                yp[:],
                                wp[:, k, m * 128:(m + 1) * 128],
                                ot[:, k, c * 264:(c + 1) * 264],
                                start=(k == 0), stop=(k == ND - 1),
                            )
                        yst = ystp.tile([128, 264], BF)
                        nc.scalar.copy(yst[:], yp[:])
                        nc.sync.dma_start(
                            yt_ext[b, m * 128:(m + 1) * 128, c * 264:(c + 1) * 264],
                            yst[:],
                        )
    if split_waits:
        _split_multi_waits(nc)
    return nc


_CACHE = {}


def _get_bass():
    if "nc" not in _CACHE:
        _CACHE["nc"] = build_bass()
    return _CACHE["nc"]


def kernel(x, Wq, Wk, Wv, Wp, bp, t_h=8, t_w=8, s_h=20, s_w=20, _trace=False):
    assert int(t_h) * int(t_w) == 64 and int(s_h) * int(s_w) == 400
    x = np.asarray(x, np.float32)
    assert x.shape == (B, L, D), x.shape

    xt = np.ascontiguousarray(
        x.reshape(NCORES, BPC, L, D).transpose(0, 1, 3, 2)
    ).astype(ml_dtypes.bfloat16)
    wbf = {
        n: np.ascontiguousarray(np.asarray(w, np.float32)).astype(
            ml_dtypes.bfloat16
        )
        for n, w in (("wq", Wq), ("wk", Wk), ("wv", Wv), ("wp", Wp))
    }

    nc = _get_bass()
    in_maps = [{"xt": xt[i], **wbf} for i in range(NCORES)]
    res = run_bass_kernel_spmd(
        nc, in_maps, core_ids=list(range(NCORES)), trace=_trace
    )
    y = np.stack(
        [np.asarray(res.results[i]["yt"], np.float32) for i in range(NCORES)]
    )
    y = y.transpose(0, 1, 3, 2).reshape(B, L, D)
    y = y + np.asarray(bp, np.float32)[None, None, :]
    if _trace:
        _CACHE["last_result"] = res
    return y.astype(np.float32)
